# revision 11
# baseline (speedup 1.0000x reference)
"""BiLSTM-CRF NLL kernel for 8 trn2 NeuronCores (SPMD, role-via-data).

Structure (one bass program, identical on all cores; cores 0/1 form the
working pair — fwd/bwd roles are set purely by per-core input data; pairs
(2,3),(4,5),(6,7) run duplicates):

  1. WX1 = W1_aug @ x_aug^T  (fp32r matmuls, [4096 x 2048] per core)
  2. layer-1 LSTM chain, 2048 sequential steps (own direction, own-time
     order; the bwd core consumes host-reversed inputs)
  3. pair AllReduce(add) on H1 -> partner states recovered as (sum - own),
     time-reversed on the fly (negative-stride reads)
  4. WX2 precompute, layer-2 chain, pair AllReduce on H2
  5. emissions E = Wout_aug @ [own H2; partner H2; 1]
  6. CRF forward pass in exp space on the PE (p' = Texp @ p * exp(emit)),
     periodic rescaling; logZ output. Gold score is host-side.

Per-step recurrence: gates[4096] = Whh @ h via 1024 col-tiled self-loading
fp32 matmuls (lhsT [128,32], N=1) accumulating into psum [128,32];
sigmoid/tanh on ACT; cell update on DVE.
"""
import numpy as np

SEQ = 2048
ELMO = 1024
POS = 64
HID = 1024
NTAG = 48
START_IDX = 0
END_IDX = 1
D1 = ELMO + POS          # 1088
D1P = 1152               # padded (9*128): +1 bias row
KC1 = 9
D2 = 2 * HID             # 2048
D2P = 2176               # padded (17*128): +1 bias row
KC2 = 17
G = 4 * HID              # 4096
BLK = 64                 # steps per window block
NBLK = SEQ // BLK        # 32
WXT = SEQ + BLK          # padded time dim of WX (prefetch overrun)
RS = 8                   # CRF rescale cadence

_PROG = None


def _g3(ap):
    return ap.rearrange("p (c o) -> p c o", o=1)


def _build_program():
    import concourse.bass as bass
    import concourse.tile as tile
    from concourse import bacc, mybir

    f32 = mybir.dt.float32
    f32r = mybir.dt.float32r
    AF = mybir.ActivationFunctionType

    nc = bacc.Bacc("TRN2", target_bir_lowering=False, debug=False, num_devices=8)

    xT = nc.dram_tensor("xT", [KC1, 128, SEQ], f32r, kind="ExternalInput").ap()
    w1T = nc.dram_tensor("w1T", [KC1, 128, G], f32r, kind="ExternalInput").ap()
    whh1 = nc.dram_tensor("whh1", [128, 8 * G], f32, kind="ExternalInput").ap()
    w2T = nc.dram_tensor("w2T", [KC2, 128, G], f32r, kind="ExternalInput").ap()
    whh2 = nc.dram_tensor("whh2", [128, 8 * G], f32, kind="ExternalInput").ap()
    woutT = nc.dram_tensor("woutT", [KC2, 128, NTAG], f32r, kind="ExternalInput").ap()
    texpT = nc.dram_tensor("texpT", [NTAG, NTAG], f32, kind="ExternalInput").ap()
    eend = nc.dram_tensor("eend", [NTAG, 1], f32, kind="ExternalInput").ap()
    p0 = nc.dram_tensor("p0", [NTAG, 1], f32, kind="ExternalInput").ap()
    biasr = nc.dram_tensor("biasr", [128, SEQ], f32r, kind="ExternalInput").ap()

    emis_o = nc.dram_tensor("emis", [NTAG, SEQ], f32, kind="ExternalOutput").ap()
    logz_o = nc.dram_tensor("logz", [1, 1], f32, kind="ExternalOutput").ap()

    with tile.TileContext(nc) as tc:
        _trace(tc, bass, mybir, f32, f32r, AF,
               xT, w1T, whh1, w2T, whh2, woutT, texpT, eend, p0, biasr,
               emis_o, logz_o)
    nc.compile()
    return nc


def _trace(tc, bass, mybir, f32, f32r, AF,
           xT, w1T, whh1, w2T, whh2, woutT, texpT, eend, p0, biasr,
           emis_o, logz_o):
    from contextlib import ExitStack
    nc = tc.nc
    ds, ts = bass.ds, bass.ts

    with ExitStack() as ctx:
        dram = ctx.enter_context(tc.tile_pool(name="dram", bufs=1, space="DRAM"))
        WX = dram.tile([32, 128, WXT], f32)
        H1loc = dram.tile([8, 128, SEQ], f32)
        H1sum = dram.tile([8, 128, SEQ], f32)
        H2loc = dram.tile([8, 128, SEQ], f32)
        H2sum = dram.tile([8, 128, SEQ], f32)

        state = ctx.enter_context(tc.tile_pool(name="state", bufs=1))
        h = state.tile([128, 8], f32)
        c = state.tile([128, 8], f32)
        t1 = state.tile([128, 8], f32)
        t2 = state.tile([128, 8], f32)
        u = state.tile([128, 8], f32)
        gates = state.tile([128, 32], f32)

        pps = ctx.enter_context(tc.tile_pool(name="pps", bufs=1, space="PSUM"))
        gpsum = pps.tile([128, 32], f32)

        # ---------------- WX precompute helper ----------------
        def wx_precompute(wT, kc, rhs_tiles, wpool, pspool, opool):
            for m in range(32):
                wt = wpool.tile([128, kc * 128], f32r, tag="wstage")
                for k in range(kc):
                    nc.gpsimd.dma_start(
                        wt[:, 128 * k:128 * k + 128],
                        wT[k, :, 128 * m:128 * m + 128])
                for n in range(SEQ // 512):
                    ps = pspool.tile([128, 512], f32, tag="wxps")
                    for k in range(kc):
                        nc.tensor.matmul(ps[:], wt[:, 128 * k:128 * k + 128],
                                         rhs_tiles[k][:, 512 * n:512 * n + 512],
                                         start=(k == 0), stop=(k == kc - 1))
                    ob = opool.tile([128, 512], f32, tag="wxob")
                    nc.vector.tensor_copy(ob[:], ps[:])
                    nc.gpsimd.dma_start(WX[m, :, 512 * n:512 * n + 512], ob[:])

        # ---------------- LSTM phase ----------------
        def lstm_phase(whh_sb, Hloc, win0, win1, ring0, ring1):
            nc.gpsimd.memset(h[:], 0.0)
            nc.gpsimd.memset(c[:], 0.0)

            def load_win(win, col_ds):
                for cb in range(32):
                    nc.gpsimd.dma_start(
                        win[:, BLK * cb:BLK * cb + BLK],
                        WX[cb, :, col_ds])

            def flush_ring(ring, col_ds):
                for q in range(8):
                    nc.gpsimd.dma_start(
                        Hloc[q, :, col_ds],
                        ring[:, BLK * q:BLK * q + BLK])

            def step(j, win, ring):
                for jj in range(32):
                    for k in range(8):
                        for g in range(4):
                            flat = (g * 32 + jj) * 8 + k
                            nc.tensor.matmul(
                                gpsum[32 * g:32 * g + 32, jj:jj + 1],
                                whh_sb[:, 32 * flat:32 * flat + 32],
                                h[:, k:k + 1],
                                start=(k == 0), stop=(k == 7),
                                tile_position=(0, 32 * g))
                wxs = win[:].rearrange("p (c t) -> p c t", t=BLK)[:, :, ds(j, 1)]
                nc.vector.tensor_add(_g3(gates[:]), _g3(gpsum[:]), wxs)
                nc.scalar.activation(gates[:, 0:24], gates[:, 0:24], AF.Sigmoid)
                nc.scalar.activation(gates[:, 24:32], gates[:, 24:32], AF.Tanh)
                nc.vector.tensor_mul(t1[:], gates[:, 8:16], c[:])
                nc.vector.tensor_mul(t2[:], gates[:, 0:8], gates[:, 24:32])
                nc.vector.tensor_add(c[:], t1[:], t2[:])
                nc.scalar.activation(u[:], c[:], AF.Tanh)
                nc.vector.tensor_mul(h[:], gates[:, 16:24], u[:])
                nc.vector.tensor_copy(
                    ring[:].rearrange("p (q t) -> p q t", t=BLK)[:, :, ds(j, 1)],
                    _g3(h[:]))

            def inner(win, ring):
                with tc.For_i(0, BLK, 1) as j:
                    step(j, win, ring)

            load_win(win0, ds(0, BLK))
            with tc.For_i(0, NBLK // 2, 1) as bb:
                load_win(win1, ds(bb * (2 * BLK) + BLK, BLK))
                inner(win0, ring0)
                flush_ring(ring0, ds(bb * (2 * BLK), BLK))
                load_win(win0, ds(bb * (2 * BLK) + 2 * BLK, BLK))
                inner(win1, ring1)
                flush_ring(ring1, ds(bb * (2 * BLK) + BLK, BLK))

        # ---------------- partner-chunk rhs builder ----------------
        def build_rhs(pool, Hown, Hsum, tmppool):
            tiles = []
            for q in range(8):
                tl = pool.tile([128, SEQ], f32r, tag=f"rhs{q}")
                nc.gpsimd.dma_start(tl[:], Hown[q].bitcast(f32r))
                tiles.append(tl)
            for q in range(8):
                tl = pool.tile([128, SEQ], f32r, tag=f"rhs{8 + q}")
                for b in range(SEQ // 512):
                    ts_ = tmppool.tile([128, 512], f32, tag="rtmp_s")
                    to_ = tmppool.tile([128, 512], f32, tag="rtmp_o")
                    nc.gpsimd.dma_start(ts_[:], Hsum[q, :, 512 * b:512 * b + 512])
                    nc.gpsimd.dma_start(to_[:], Hown[q, :, 512 * b:512 * b + 512])
                    nc.vector.tensor_sub(
                        tl[:, SEQ - 512 * (b + 1):SEQ - 512 * b],
                        ts_[:, ::-1], to_[:, ::-1])
                tiles.append(tl)
            bt = pool.tile([128, SEQ], f32r, tag="rhs16")
            nc.gpsimd.dma_start(bt[:], biasr[:])
            tiles.append(bt)
            return tiles

        # ================ stages 1-2: recurrent phases ================
        with tc.tile_pool(name="recur", bufs=1) as rp:
            win0 = rp.tile([128, 32 * BLK], f32)
            win1 = rp.tile([128, 32 * BLK], f32)
            ring0 = rp.tile([128, 8 * BLK], f32)
            ring1 = rp.tile([128, 8 * BLK], f32)

            # zero the WX prefetch-overrun block (read, never consumed)
            nc.gpsimd.memset(win0[:], 0.0)
            for cb in range(32):
                nc.gpsimd.dma_start(WX[cb, :, SEQ:WXT], win0[:, 0:BLK])

            with tc.tile_pool(name="xin", bufs=1) as xin, \
                 tc.tile_pool(name="wstage", bufs=2) as wpool, \
                 tc.tile_pool(name="wxps", bufs=2, space="PSUM") as pspool, \
                 tc.tile_pool(name="wxob", bufs=3) as opool:
                x_tiles = []
                for k in range(KC1):
                    tl = xin.tile([128, SEQ], f32r, tag=f"x{k}")
                    nc.gpsimd.dma_start(tl[:], xT[k])
                    x_tiles.append(tl)
                wx_precompute(w1T, KC1, x_tiles, wpool, pspool, opool)

            with tc.tile_pool(name="whhA", bufs=1) as whhp:
                whh_sb = whhp.tile([128, 8 * G], f32)
                nc.gpsimd.dma_start(whh_sb[:], whh1[:])
                lstm_phase(whh_sb, H1loc, win0, win1, ring0, ring1)

            nc.gpsimd.collective_compute(
                "AllReduce", mybir.AluOpType.add,
                replica_groups=[[0, 1], [2, 3], [4, 5], [6, 7]],
                ins=[H1loc.opt()], outs=[H1sum.opt()])

            with tc.tile_pool(name="hrhs", bufs=1) as hpool, \
                 tc.tile_pool(name="rtmp", bufs=2) as tmppool, \
                 tc.tile_pool(name="wstage2", bufs=2) as wpool2, \
                 tc.tile_pool(name="wxps2", bufs=2, space="PSUM") as pspool2, \
                 tc.tile_pool(name="wxob2", bufs=3) as opool2:
                rhs2 = build_rhs(hpool, H1loc, H1sum, tmppool)
                wx_precompute(w2T, KC2, [t[:] for t in rhs2],
                              wpool2, pspool2, opool2)

            with tc.tile_pool(name="whhB", bufs=1) as whhp2:
                whh_sb2 = whhp2.tile([128, 8 * G], f32)
                nc.gpsimd.dma_start(whh_sb2[:], whh2[:])
                lstm_phase(whh_sb2, H2loc, win0, win1, ring0, ring1)

            nc.gpsimd.collective_compute(
                "AllReduce", mybir.AluOpType.add,
                replica_groups=[[0, 1], [2, 3], [4, 5], [6, 7]],
                ins=[H2loc.opt()], outs=[H2sum.opt()])

        # ================ stage 3: emissions + CRF ================
        crf = ctx.enter_context(tc.tile_pool(name="crf", bufs=1))
        E_sb = crf.tile([NTAG, SEQ], f32)
        eem = crf.tile([NTAG, SEQ], f32)
        texp_sb = crf.tile([NTAG, NTAG], f32)
        p_sb = crf.tile([NTAG, 1], f32)
        ls_sb = crf.tile([NTAG, 1], f32)      # scalar state at partition 32
        r_sb = crf.tile([NTAG, 1], f32)       # reciprocal at partition 32
        lg_sb = crf.tile([NTAG, 1], f32)      # log at partition 32
        onesr = crf.tile([NTAG, NTAG], f32)   # ones row for PE broadcast
        ones_sb = crf.tile([NTAG, 1], f32)
        eend_sb = crf.tile([NTAG, 1], f32)
        q_sb = crf.tile([NTAG, 1], f32)
        lz_sb = crf.tile([1, 1], f32)

        with tc.tile_pool(name="erhs", bufs=1) as epool, \
             tc.tile_pool(name="etmp", bufs=2) as etmp, \
             tc.tile_pool(name="eps", bufs=2, space="PSUM") as epsp, \
             tc.tile_pool(name="ewst", bufs=1) as ewst:
            rhsE = [t[:] for t in build_rhs(epool, H2loc, H2sum, etmp)]
            wt = ewst.tile([128, KC2 * NTAG], f32r)
            for k in range(KC2):
                nc.gpsimd.dma_start(
                    wt[:, NTAG * k:NTAG * k + NTAG], woutT[k])
            for n in range(SEQ // 512):
                ps = epsp.tile([NTAG, 512], f32, tag="eps")
                for k in range(KC2):
                    nc.tensor.matmul(ps[:], wt[:, NTAG * k:NTAG * k + NTAG],
                                     rhsE[k][:, 512 * n:512 * n + 512],
                                     start=(k == 0), stop=(k == KC2 - 1))
                nc.vector.tensor_copy(E_sb[:, 512 * n:512 * n + 512], ps[:])
        nc.gpsimd.dma_start(emis_o[:], E_sb[:])

        nc.scalar.activation(eem[:], E_sb[:], AF.Exp)
        nc.gpsimd.dma_start(texp_sb[:], texpT[:])
        nc.gpsimd.dma_start(p_sb[:], p0[:])
        nc.gpsimd.dma_start(eend_sb[:], eend[:])
        nc.gpsimd.memset(ls_sb[:], 0.0)
        nc.gpsimd.memset(ones_sb[:], 1.0)

        nc.gpsimd.memset(onesr[:], 1.0)
        J0 = 32  # live tag, 32-aligned partition (MM operand requirement)
        with tc.tile_pool(name="crfps", bufs=1, space="PSUM") as cpsp:
            pq = cpsp.tile([NTAG, 1], f32)
            rb = cpsp.tile([NTAG, 1], f32, tag="rb")
            for t in range(SEQ):
                nc.tensor.matmul(pq[:], texp_sb[:], p_sb[:], start=True, stop=True)
                nc.vector.tensor_mul(p_sb[:], pq[:], eem[:, t:t + 1])
                if t % RS == RS - 1:
                    sl = slice(J0, J0 + 1)
                    nc.vector.reciprocal(r_sb[sl, :], p_sb[sl, :])
                    nc.scalar.activation(lg_sb[sl, :], p_sb[sl, :], AF.Ln)
                    nc.vector.tensor_add(ls_sb[sl, :], ls_sb[sl, :], lg_sb[sl, :])
                    # PE broadcast: rb[:, 0] = r for all 48 partitions
                    nc.tensor.matmul(rb[:], onesr[J0:J0 + 1, :], r_sb[sl, :],
                                     start=True, stop=True)
                    nc.vector.tensor_mul(p_sb[:], p_sb[:], rb[:])
            nc.vector.tensor_mul(q_sb[:], p_sb[:], eend_sb[:])
            sps = cpsp.tile([1, 1], f32, tag="sps")
            nc.tensor.matmul(sps[:], ones_sb[:], q_sb[:], start=True, stop=True)
            nc.scalar.activation(lz_sb[:], sps[:], AF.Ln)
            lsb = cpsp.tile([1, 1], f32, tag="lsb")
            nc.tensor.matmul(lsb[:], onesr[J0:J0 + 1, 0:1], ls_sb[J0:J0 + 1, :],
                             start=True, stop=True)
            nc.vector.tensor_add(lz_sb[:], lz_sb[:], lsb[:])
        nc.gpsimd.dma_start(logz_o[:], lz_sb[:])


# ---------------------------------------------------------------------------
# host side
# ---------------------------------------------------------------------------

_GPERM = None
_BIASR = None


def _bias_rhs():
    global _BIASR
    if _BIASR is None:
        b = np.zeros((128, SEQ), np.float32)
        b[0] = 1.0
        _BIASR = b
    return _BIASR


def _gate_perm():
    global _GPERM
    if _GPERM is None:
        a = np.arange(1024)
        _GPERM = np.concatenate([a, 1024 + a, 3072 + a, 2048 + a])  # i f o g
    return _GPERM


def _pack_whh(Whh):
    """Whh [4096(perm'd), 1024] -> packed [128, 32768], block flat=(g*32+j)*8+k."""
    WT = np.ascontiguousarray(Whh.T).astype(np.float32)      # [1024, 4096]
    W5 = WT.reshape(8, 128, 32, 4, 32)                       # [k,p,j,g,s]
    blocks = W5.transpose(3, 2, 0, 1, 4)                     # [g,j,k,p,s]
    return np.ascontiguousarray(
        blocks.reshape(1024, 128, 32).transpose(1, 0, 2).reshape(128, 8 * 4096))


def _pack_wT(W, b, kc, mcols):
    """W [mcols, d_in], b [mcols] -> [kc, 128, mcols] of [W | b | 0pad]^T."""
    d_in = W.shape[1]
    out = np.zeros((kc * 128, mcols), np.float32)
    out[:d_in] = W.T
    out[d_in] = b
    return np.ascontiguousarray(out.reshape(kc, 128, mcols))


def kernel(sentence, speech_tags, tags, emb_table, lstm_params, W_out, b_out,
           transitions):
    global _PROG
    from concourse.bass_utils import run_bass_kernel_spmd

    sentence = np.asarray(sentence, np.float32)
    speech_tags = np.asarray(speech_tags)
    tags = np.asarray(tags)
    emb_table = np.asarray(emb_table, np.float32)
    W_out = np.asarray(W_out, np.float32)
    b_out = np.asarray(b_out, np.float32)
    transitions = np.asarray(transitions, np.float32)
    perm = _gate_perm()

    x = np.concatenate([sentence, emb_table[speech_tags]], axis=1)  # [T, 1088]

    def xT_of(xd):
        xa = np.zeros((D1P, SEQ), np.float32)
        xa[:D1] = xd.T
        xa[D1] = 1.0
        return np.ascontiguousarray(xa.reshape(KC1, 128, SEQ))

    maps = []
    for role in range(2):
        (Wih_f, Whh_f, bih_f, bhh_f, Wih_b, Whh_b, bih_b, bhh_b) = lstm_params[0]
        (Wih2_f, Whh2_f, bih2_f, bhh2_f, Wih2_b, Whh2_b, bih2_b, bhh2_b) = lstm_params[1]
        if role == 0:
            Wih, Whh, bih, bhh = Wih_f, Whh_f, bih_f, bhh_f
            Wih2, Whh2, bih2, bhh2 = Wih2_f, Whh2_f, bih2_f, bhh2_f
            xd = x
            own_sl, par_sl = slice(0, HID), slice(HID, 2 * HID)
        else:
            Wih, Whh, bih, bhh = Wih_b, Whh_b, bih_b, bhh_b
            Wih2, Whh2, bih2, bhh2 = Wih2_b, Whh2_b, bih2_b, bhh2_b
            xd = x[::-1]
            own_sl, par_sl = slice(HID, 2 * HID), slice(0, HID)

        Wih = np.asarray(Wih, np.float32)[perm]
        b1 = (np.asarray(bih, np.float32) + np.asarray(bhh, np.float32))[perm]
        Whh = np.asarray(Whh, np.float32)[perm]
        Wih2 = np.asarray(Wih2, np.float32)[perm]
        b2 = (np.asarray(bih2, np.float32) + np.asarray(bhh2, np.float32))[perm]
        Whh2 = np.asarray(Whh2, np.float32)[perm]

        W2ro = np.concatenate([Wih2[:, own_sl], Wih2[:, par_sl]], axis=1)
        Wout_ro = np.concatenate([W_out[:, own_sl], W_out[:, par_sl]], axis=1)

        m = {
            "xT": xT_of(xd),
            "w1T": _pack_wT(Wih, b1, KC1, G),
            "whh1": _pack_whh(Whh),
            "w2T": _pack_wT(W2ro, b2, KC2, G),
            "whh2": _pack_whh(Whh2),
            "woutT": _pack_wT(Wout_ro, b_out, KC2, NTAG),
            "texpT": np.ascontiguousarray(np.exp(transitions).T),
            "eend": np.ascontiguousarray(np.exp(transitions[END_IDX])[:, None]),
            "p0": np.eye(NTAG, 1, -START_IDX, dtype=np.float32),
            "biasr": _bias_rhs(),
        }
        maps.append(m)

    in_maps = [maps[i % 2] for i in range(8)]

    if _PROG is None:
        _PROG = _build_program()
    res = run_bass_kernel_spmd(_PROG, in_maps, list(range(8)))
    out = res.results[0]
    E = out["emis"]                       # [48, 2048]
    logZ = np.float32(out["logz"][0, 0])

    tags_ext = np.concatenate([[START_IDX], tags])
    gold = np.float32(transitions[tags_ext[1:], tags_ext[:-1]].sum()
                      + transitions[END_IDX, tags[-1]]
                      + E[tags, np.arange(SEQ)].sum())
    return np.float32(logZ - gold)
